# revision 9
# baseline (speedup 1.0000x reference)
"""ChannelBlockImportanceGate kernel for 8 Trainium2 NeuronCores.

Computes, per (b, c) slice of features [8, 256, 132, 132] f32:
  scores = block-sum of |x| over 8x8 blocks (17x17 grid, zero-padded edges)
  top-72 blocks (ties -> lowest index, matching jax.lax.top_k)
  output = per-pixel {0,1} mask upsampled 8x8 (cropped to 132x132)

The straight-through soft term of the reference cancels in the forward
pass (hard - sg(soft) + soft == hard up to ~1ulp), so the output is the
hard mask.

Sharding: purely data parallel. 2048 (b,c) slices -> 256 per core.
Per core: 2 groups of 128 slices; each slice occupies one SBUF
partition so pooling/topk are per-partition ops with no cross-partition
traffic. Top-72 uses 9 rounds of DVE max8 + match_replace(-2^100).

v5 structure (trace-driven; the Vector engine's serial chain paces the
second group's stores, so work is spread over all four engines):
 - Five row chunks (4x32 + 1x4 rows) so the last-loaded chunk's W-pool
   is 0.75us, shortening last-load -> last-group-mask latency.
 - Chunks 0-1 of each group take an offload W-pool path: Scalar does
   an in-place ACTIVATE Abs over the chunk, GpSimd runs the 8->1
   column-sum as a 3-level tensor_tensor add tree (the Pool engine ISA
   has no abs or reduce, and runs ~7.4us/chunk vs DVE's 4.4us — but in
   parallel). Chunks 2-4 stay on DVE tensor_reduce(|x|). This takes
   ~9us/group off the Vector critical chain.
 - H-pool is a 3-level tensor_tensor add tree (~2.7us) instead of one
   strided 8:1 tensor_reduce (3.8us).
 - A broadcast copy from group g's scores into group g+1's wsum (only
   the DVE-pooled chunk ranges) right after topk gives the Tile
   list-scheduler a dependency that stops it hoisting g+1's 4.4us
   W-pool reduces into g's topk max8/match_replace latency chain (in
   the v1 trace that delayed the first store by ~25us, leaving DMA
   idle 61->83us). The GpSimd-pooled ranges stay unconstrained — that
   engine is otherwise idle, so early execution there is free.
 - match_replace writes -2^100, so mask-and-W-expand is ONE Scalar
   ACTIVATE per row range: relu(score * -2^-100) is exactly 1.0 for
   replaced (top-72) entries and exactly 0.0 otherwise, written
   directly as a W-expanded row strip [128, 17, 136] via a stride-0
   broadcast read. Vector's post-topk work is just the dep copy.
 - Upsample 8x in H in place over a feature chunk = one 3D stride-0
   broadcast Scalar copy per chunk; each chunk's store is issued right
   afterwards from the Scalar engine's own HWDGE queue (loads live on
   the Sync queue, so a store waiting for its mask can never
   head-of-line-block a load). Store descriptors stay at the fat
   16.9KB-per-partition line-rate size.
"""

import numpy as np

B, C, H, W = 8, 256, 132, 132
HW = H * W            # 17424
NB = 17               # 8x8 blocks per side (132 padded to 136)
NBLK = NB * NB        # 289
KEEP = 72             # round(289 * 0.25)
N_CORES = 8
S = (B * C) // N_CORES  # 256 slices per core
ROW_CHUNKS = ((0, 32), (32, 64), (64, 96), (96, 128), (128, 132))
NEG = -(2.0 ** 100)
MSCALE = -(2.0 ** -100)
WE = NB * 8           # 136 cols per W-expanded row strip

_prog_cache = {}


def _build_program():
    import concourse.bacc as bacc
    import concourse.mybir as mybir
    import concourse.tile as tile

    f32 = mybir.dt.float32
    X = mybir.AxisListType.X
    ADD = mybir.AluOpType.add
    RELU = mybir.ActivationFunctionType.Relu
    ABS = mybir.ActivationFunctionType.Abs

    nc = bacc.Bacc("TRN2", debug=False, num_devices=N_CORES)
    x = nc.dram_tensor("x", (S, HW), f32, kind="ExternalInput")
    y = nc.dram_tensor("y", (S, HW), f32, kind="ExternalOutput")

    ngroups = S // 128

    with tile.TileContext(nc) as tc:
        with (
            tc.tile_pool(name="big", bufs=2) as bigp,
            tc.tile_pool(name="med", bufs=2) as medp,
            tc.tile_pool(name="one", bufs=1) as onep,
            tc.tile_pool(name="small", bufs=2) as smallp,
        ):
            # wsum tiles for all groups upfront so group g's epilogue can
            # write the ordering dep into group g+1's tile.
            wsums = [medp.tile([128, H * NB], f32, name=f"wsum_g{g}",
                               tag="wsum") for g in range(ngroups)]
            # gpsimd add-tree scratch, shared by all offloaded chunks (the
            # trees run back to back on the Pool engine's FIFO anyway).
            t1 = onep.tile([128, 32 * 16 * 4], f32, name="t1", tag="t1")
            t2 = onep.tile([128, 32 * 16 * 2], f32, name="t2", tag="t2")
            t13 = t1.rearrange("p (r q c) -> p r q c", q=16, c=4)
            t23 = t2.rearrange("p (r q c) -> p r q c", q=16, c=2)
            for g in range(ngroups):
                p0 = g * 128
                chunks = []
                for k, (r0, r1) in enumerate(ROW_CHUNKS):
                    ch = bigp.tile([128, (r1 - r0) * W], f32,
                                   name=f"ch_g{g}k{k}", tag=f"chunk{k}")
                    nc.sync.dma_start(out=ch[:, :],
                                      in_=x[p0:p0 + 128, r0 * W:r1 * W])
                    chunks.append(ch)

                # W-pool: per image row, |x| summed over 8-col groups
                # (16 full groups + one 4-col partial group).
                # Chunks 0-1: Scalar abs (in place) + GpSimd add tree.
                # Chunks 2-4: DVE tensor_reduce with the abs flag.
                wsum = wsums[g]
                ws3 = wsum.rearrange("p (r t) -> p r t", t=NB)
                for k, (r0, r1) in enumerate(ROW_CHUNKS):
                    v = chunks[k].rearrange("p (r w) -> p r w", w=W)
                    if k < 2:
                        nc.scalar.activation(out=chunks[k][:, :],
                                             in_=chunks[k][:, :], func=ABS)
                        a4 = v[:, :, 0:128].rearrange(
                            "p r (q c) -> p r q c", c=8)
                        nc.gpsimd.tensor_tensor(
                            out=t13, in0=a4[:, :, :, 0:4],
                            in1=a4[:, :, :, 4:8], op=ADD)
                        nc.gpsimd.tensor_tensor(
                            out=t23, in0=t13[:, :, :, 0:2],
                            in1=t13[:, :, :, 2:4], op=ADD)
                        nc.gpsimd.tensor_tensor(
                            out=ws3[:, r0:r1, 0:16], in0=t23[:, :, :, 0],
                            in1=t23[:, :, :, 1], op=ADD)
                    else:
                        nc.vector.tensor_reduce(
                            out=ws3[:, r0:r1, 0:16],
                            in_=v[:, :, 0:128].rearrange(
                                "p r (q c) -> p r q c", c=8),
                            axis=X, op=ADD, apply_absolute_value=True)
                    nc.vector.tensor_reduce(
                        out=ws3[:, r0:r1, 16:17],
                        in_=v[:, :, 128:132],
                        axis=X, op=ADD, apply_absolute_value=True)

                # H-pool: sum row sums over 8-row groups. Full 128 rows as a
                # 3-level pairwise add tree, the 4-row remainder reduced
                # directly -> scores [128, 289], layout h*17 + w.
                ht1 = onep.tile([128, 64 * NB], f32, name=f"ht1_g{g}",
                                tag="ht1")
                ht2 = onep.tile([128, 32 * NB], f32, name=f"ht2_g{g}",
                                tag="ht2")
                scores = smallp.tile([128, NBLK], f32,
                                     name=f"scores_g{g}", tag="scores")
                sc3 = scores.rearrange("p (h t) -> p h t", t=NB)
                w4 = ws3[:, 0:128, :].rearrange("p (a b) t -> p a b t", b=2)
                h13 = ht1.rearrange("p (r t) -> p r t", t=NB)
                nc.vector.tensor_tensor(out=h13[:, :, :],
                                        in0=w4[:, :, 0, :],
                                        in1=w4[:, :, 1, :], op=ADD)
                h14 = h13.rearrange("p (a b) t -> p a b t", b=2)
                h23 = ht2.rearrange("p (r t) -> p r t", t=NB)
                nc.vector.tensor_tensor(out=h23[:, :, :],
                                        in0=h14[:, :, 0, :],
                                        in1=h14[:, :, 1, :], op=ADD)
                h24 = h23.rearrange("p (a b) t -> p a b t", b=2)
                nc.vector.tensor_tensor(out=sc3[:, 0:16, :],
                                        in0=h24[:, :, 0, :],
                                        in1=h24[:, :, 1, :], op=ADD)
                nc.vector.tensor_reduce(
                    out=sc3[:, 16:17, :],
                    in_=ws3[:, 128:132, :].rearrange("p r t -> p t r"),
                    axis=X, op=ADD)

                # Top-72 per partition: 9 rounds of DVE max8 + match_replace.
                # match_replace replaces the first unmatched occurrence, so
                # ties resolve to the lowest index like jax.lax.top_k.
                for it in range(KEEP // 8):
                    m8 = smallp.tile([128, 8], f32,
                                     name=f"m8_g{g}i{it}", tag="m8")
                    nc.vector.max(out=m8[:, :], in_=scores[:, :])
                    nc.vector.match_replace(out=scores[:, :],
                                            in_to_replace=m8[:, :],
                                            in_values=scores[:, :],
                                            imm_value=NEG)

                # Ordering dep: a broadcast copy over the next group's wsum
                # ranges of the DVE-pooled chunks (2-4) so the scheduler
                # cannot start the next group's big W-pool reduces before
                # this group's topk has finished. The GpSimd-pooled ranges
                # (chunks 0-1) stay unconstrained on purpose.
                if g + 1 < ngroups:
                    nc.vector.tensor_copy(
                        out=wsums[g + 1][0:1, 1088:2177],
                        in_=scores[0:1, 288:289].broadcast_to((1, 1089)))

                # Mask + W-expand on Scalar: relu(score * -2^-100) is 1.0
                # exactly for replaced (top-72) entries, 0.0 otherwise.
                # One op per row range over the [17 blocks x 8] layout
                # (cols 132:136 get written too; they are never read).
                # Split so the rows chunk 0 needs are produced first.
                wexp = onep.tile([128, NB * WE], f32, name=f"wexp_g{g}",
                                 tag="wexp")
                we3 = wexp.rearrange("p (h w) -> p h w", w=WE)

                def wexp_rows(h0, h1):
                    outW = (we3[:, h0:h1, :]
                            .rearrange("p h (t c) -> p h t c", c=8))
                    inW = (sc3[:, h0:h1, :].unsqueeze(3)
                           .broadcast_to((128, h1 - h0, NB, 8)))
                    nc.scalar.activation(out=outW, in_=inW, func=RELU,
                                         scale=MSCALE)

                # Upsample 8x in H in place over the feature chunks (one 3D
                # stride-0 broadcast copy on Scalar per chunk), then store
                # from the Scalar engine's own HWDGE queue.
                wexp_rows(0, 4)
                for k, (r0, r1) in enumerate(ROW_CHUNKS):
                    if k == 1:
                        wexp_rows(4, NB)
                    ch = chunks[k]
                    v = ch.rearrange("p (r w) -> p r w", w=W)
                    hg0 = r0 // 8
                    nfull = (min(r1, 128) - r0) // 8
                    nr = nfull * 8
                    if nfull:
                        outA = v[:, 0:nr, :].rearrange(
                            "p (h r) w -> p h r w", r=8)
                        inA = (we3[:, hg0:hg0 + nfull, 0:W].unsqueeze(2)
                               .broadcast_to((128, nfull, 8, W)))
                        nc.scalar.copy(out=outA, in_=inA)
                    if r1 > 128:  # rows 128..131: the 4-row partial hgroup
                        a = max(0, 128 - r0)
                        outB = v[:, a:a + 4, :]
                        inB = we3[:, 16:17, 0:W].broadcast_to((128, 4, W))
                        nc.scalar.copy(out=outB, in_=inB)
                    nc.scalar.dma_start(out=y[p0:p0 + 128, r0 * W:r1 * W],
                                        in_=ch[:, :])
    nc.compile()
    return nc


def _ensure_ntff_hook_module():
    """bass_utils' trace path does `from antenv.axon_hooks import
    get_axon_ntff_profile_hook` — a module this image doesn't ship.
    Register an equivalent (ctypes into libaxon_pjrt.so, mirroring
    trn_boot._ntff_profile_via_ctypes) so BASS_TRACE=1 works; degrade
    to a None hook (trace skipped) when unavailable."""
    import sys
    import types

    try:
        import antenv.axon_hooks  # noqa: F401
        return
    except Exception:
        pass

    hook = None
    try:
        import contextlib
        import ctypes

        so_path = "/opt/axon/libaxon_pjrt.so"
        lib = ctypes.CDLL(so_path)
        if hasattr(lib, "axon_start_nrt_profile"):
            lib.axon_start_nrt_profile.argtypes = [
                ctypes.POINTER(ctypes.c_int64), ctypes.c_size_t]
            lib.axon_start_nrt_profile.restype = ctypes.c_int64
            lib.axon_stop_nrt_profile.argtypes = [ctypes.c_char_p]
            lib.axon_stop_nrt_profile.restype = ctypes.c_int64

            @contextlib.contextmanager
            def _hook(output_dir, device_ids):
                import jax
                jax.devices()
                if device_ids:
                    ids = (ctypes.c_int64 * len(device_ids))(*device_ids)
                    rc = lib.axon_start_nrt_profile(ids, len(device_ids))
                else:
                    rc = lib.axon_start_nrt_profile(None, 0)
                if rc != 0:
                    raise RuntimeError(f"axon_start_nrt_profile rc={rc}")
                try:
                    yield
                finally:
                    n = lib.axon_stop_nrt_profile(str(output_dir).encode())
                    print(f"ntff profile: {n} file(s) -> {output_dir}",
                          file=sys.stderr)

            hook = _hook
    except Exception:
        hook = None

    mod = types.ModuleType("antenv.axon_hooks")
    mod.get_axon_ntff_profile_hook = lambda: hook
    mod.set_axon_ntff_profile_hook = lambda h: None
    sys.modules["antenv.axon_hooks"] = mod


def _get_program():
    if "nc" not in _prog_cache:
        _prog_cache["nc"] = _build_program()
    return _prog_cache["nc"]


def kernel(features, enabled):
    feats = np.asarray(features)
    if not bool(np.asarray(enabled)):
        return np.ones(feats.shape, dtype=np.float32)

    _ensure_ntff_hook_module()
    import concourse.bass_utils as _bu
    from concourse.bass_utils import run_bass_kernel_spmd

    # The trace path uploads artifacts to a shared bucket; tolerate
    # sandboxes where that fails.
    if not getattr(_bu, "_upload_patched", False):
        _orig_upload = _bu.upload_artifacts

        def _safe_upload(tmpdir):
            try:
                return _orig_upload(tmpdir)
            except Exception:
                return str(tmpdir)

        _bu.upload_artifacts = _safe_upload
        _bu._upload_patched = True

    nc = _get_program()
    flat = np.ascontiguousarray(feats.reshape(B * C, HW), dtype=np.float32)
    in_maps = [{"x": flat[i * S:(i + 1) * S]} for i in range(N_CORES)]
    res = run_bass_kernel_spmd(nc, in_maps, list(range(N_CORES)))
    _prog_cache["last_res"] = res
    out = np.concatenate([np.asarray(res.results[i]["y"])
                          for i in range(N_CORES)], axis=0)
    return out.reshape(B, C, H, W).astype(np.float32)


# revision 10
# speedup vs baseline: 1.0894x; 1.0894x over previous
"""ChannelBlockImportanceGate kernel for 8 Trainium2 NeuronCores.

Computes, per (b, c) slice of features [8, 256, 132, 132] f32:
  scores = block-sum of |x| over 8x8 blocks (17x17 grid, zero-padded edges)
  top-72 blocks (ties -> lowest index, matching jax.lax.top_k)
  output = per-pixel {0,1} mask upsampled 8x8 (cropped to 132x132)

The straight-through soft term of the reference cancels in the forward
pass (hard - sg(soft) + soft == hard up to ~1ulp), so the output is the
hard mask.

Sharding: purely data parallel. 2048 (b,c) slices -> 256 per core.
Per core: 2 groups of 128 slices; each slice occupies one SBUF
partition so pooling/topk are per-partition ops with no cross-partition
traffic. Top-72 uses 9 rounds of DVE max8 + match_replace(-2^100).

v5 structure (trace-driven; the Vector engine's serial chain paces the
second group's stores, so work is spread over all four engines):
 - Five row chunks (4x32 + 1x4 rows) so the last-loaded chunk's W-pool
   is 0.75us, shortening last-load -> last-group-mask latency.
 - Chunks 0-1 of each group take an offload W-pool path: Scalar does
   an in-place ACTIVATE Abs over the chunk, GpSimd runs the 8->1
   column-sum as a 3-level tensor_tensor add tree (the Pool engine ISA
   has no abs or reduce, and runs ~7.4us/chunk vs DVE's 4.4us — but in
   parallel). Chunks 2-4 stay on DVE tensor_reduce(|x|). This takes
   ~9us/group off the Vector critical chain.
 - H-pool is a 3-level tensor_tensor add tree (~2.7us) instead of one
   strided 8:1 tensor_reduce (3.8us).
 - A broadcast copy from group g's scores into group g+1's wsum (only
   the DVE-pooled chunk ranges) right after topk gives the Tile
   list-scheduler a dependency that stops it hoisting g+1's 4.4us
   W-pool reduces into g's topk max8/match_replace latency chain (in
   the v1 trace that delayed the first store by ~25us, leaving DMA
   idle 61->83us). The GpSimd-pooled ranges stay unconstrained — that
   engine is otherwise idle, so early execution there is free.
 - match_replace writes -2^100, so mask-and-W-expand is ONE Scalar
   ACTIVATE per row range: relu(score * -2^-100) is exactly 1.0 for
   replaced (top-72) entries and exactly 0.0 otherwise, written
   directly as a W-expanded row strip [128, 17, 136] via a stride-0
   broadcast read. Vector's post-topk work is just the dep copy.
 - Upsample 8x in H in place over a feature chunk = one 3D stride-0
   broadcast Scalar copy per chunk; each chunk's store is issued right
   afterwards from the Scalar engine's own HWDGE queue (loads live on
   the Sync queue, so a store waiting for its mask can never
   head-of-line-block a load). Store descriptors stay at the fat
   16.9KB-per-partition line-rate size.
"""

import numpy as np

B, C, H, W = 8, 256, 132, 132
HW = H * W            # 17424
NB = 17               # 8x8 blocks per side (132 padded to 136)
NBLK = NB * NB        # 289
KEEP = 72             # round(289 * 0.25)
N_CORES = 8
S = (B * C) // N_CORES  # 256 slices per core
ROW_CHUNKS = ((0, 32), (32, 64), (64, 96), (96, 128), (128, 132))
NEG = -(2.0 ** 100)
MSCALE = -(2.0 ** -100)
WE = NB * 8           # 136 cols per W-expanded row strip

_prog_cache = {}


def _build_program():
    import concourse.bacc as bacc
    import concourse.mybir as mybir
    import concourse.tile as tile

    f32 = mybir.dt.float32
    X = mybir.AxisListType.X
    ADD = mybir.AluOpType.add
    RELU = mybir.ActivationFunctionType.Relu
    ABS = mybir.ActivationFunctionType.Abs

    nc = bacc.Bacc("TRN2", debug=False, num_devices=N_CORES)
    x = nc.dram_tensor("x", (S, HW), f32, kind="ExternalInput")
    y = nc.dram_tensor("y", (S, HW), f32, kind="ExternalOutput")

    ngroups = S // 128

    with tile.TileContext(nc) as tc:
        with (
            tc.tile_pool(name="big", bufs=2) as bigp,
            tc.tile_pool(name="med", bufs=2) as medp,
            tc.tile_pool(name="one", bufs=1) as onep,
            tc.tile_pool(name="small", bufs=2) as smallp,
        ):
            # wsum tiles for all groups upfront so group g's epilogue can
            # write the ordering dep into group g+1's tile.
            wsums = [medp.tile([128, H * NB], f32, name=f"wsum_g{g}",
                               tag="wsum") for g in range(ngroups)]
            # gpsimd add-tree scratch, shared by all offloaded chunks (the
            # trees run back to back on the Pool engine's FIFO anyway).
            t1 = onep.tile([128, 32 * 16 * 4], f32, name="t1", tag="t1")
            t2 = onep.tile([128, 32 * 16 * 2], f32, name="t2", tag="t2")
            t13 = t1.rearrange("p (r q c) -> p r q c", q=16, c=4)
            t23 = t2.rearrange("p (r q c) -> p r q c", q=16, c=2)
            for g in range(ngroups):
                p0 = g * 128
                chunks = []
                for k, (r0, r1) in enumerate(ROW_CHUNKS):
                    ch = bigp.tile([128, (r1 - r0) * W], f32,
                                   name=f"ch_g{g}k{k}", tag=f"chunk{k}")
                    nc.sync.dma_start(out=ch[:, :],
                                      in_=x[p0:p0 + 128, r0 * W:r1 * W])
                    chunks.append(ch)

                # W-pool: per image row, |x| summed over 8-col groups
                # (16 full groups + one 4-col partial group).
                # Chunks 0-1: Scalar abs (in place) + GpSimd add tree.
                # Chunks 2-4: DVE tensor_reduce with the abs flag.
                wsum = wsums[g]
                ws3 = wsum.rearrange("p (r t) -> p r t", t=NB)
                for k, (r0, r1) in enumerate(ROW_CHUNKS):
                    v = chunks[k].rearrange("p (r w) -> p r w", w=W)
                    if k < 2 and g == ngroups - 1:
                        # Offload path, last group only: its pools run after
                        # the loads are mostly done, so the GpSimd SBUF
                        # traffic (~480GB/s during a tree) doesn't steal
                        # load bandwidth the way first-group offload did.
                        # Abs only cols 0:128 (what the tree consumes), so
                        # the 4-col partial reduce below reads untouched
                        # data and needs no cross-engine dependency.
                        nc.scalar.activation(out=v[:, :, 0:128],
                                             in_=v[:, :, 0:128], func=ABS)
                        a4 = v[:, :, 0:128].rearrange(
                            "p r (q c) -> p r q c", c=8)
                        nc.gpsimd.tensor_tensor(
                            out=t13, in0=a4[:, :, :, 0:4],
                            in1=a4[:, :, :, 4:8], op=ADD)
                        nc.gpsimd.tensor_tensor(
                            out=t23, in0=t13[:, :, :, 0:2],
                            in1=t13[:, :, :, 2:4], op=ADD)
                        nc.gpsimd.tensor_tensor(
                            out=ws3[:, r0:r1, 0:16], in0=t23[:, :, :, 0],
                            in1=t23[:, :, :, 1], op=ADD)
                    else:
                        nc.vector.tensor_reduce(
                            out=ws3[:, r0:r1, 0:16],
                            in_=v[:, :, 0:128].rearrange(
                                "p r (q c) -> p r q c", c=8),
                            axis=X, op=ADD, apply_absolute_value=True)
                    nc.vector.tensor_reduce(
                        out=ws3[:, r0:r1, 16:17],
                        in_=v[:, :, 128:132],
                        axis=X, op=ADD, apply_absolute_value=True)

                # H-pool: sum row sums over 8-row groups. Full 128 rows as a
                # 3-level pairwise add tree, the 4-row remainder reduced
                # directly -> scores [128, 289], layout h*17 + w.
                ht1 = onep.tile([128, 64 * NB], f32, name=f"ht1_g{g}",
                                tag="ht1")
                ht2 = onep.tile([128, 32 * NB], f32, name=f"ht2_g{g}",
                                tag="ht2")
                scores = smallp.tile([128, NBLK], f32,
                                     name=f"scores_g{g}", tag="scores")
                sc3 = scores.rearrange("p (h t) -> p h t", t=NB)
                w4 = ws3[:, 0:128, :].rearrange("p (a b) t -> p a b t", b=2)
                h13 = ht1.rearrange("p (r t) -> p r t", t=NB)
                nc.vector.tensor_tensor(out=h13[:, :, :],
                                        in0=w4[:, :, 0, :],
                                        in1=w4[:, :, 1, :], op=ADD)
                h14 = h13.rearrange("p (a b) t -> p a b t", b=2)
                h23 = ht2.rearrange("p (r t) -> p r t", t=NB)
                nc.vector.tensor_tensor(out=h23[:, :, :],
                                        in0=h14[:, :, 0, :],
                                        in1=h14[:, :, 1, :], op=ADD)
                h24 = h23.rearrange("p (a b) t -> p a b t", b=2)
                nc.vector.tensor_tensor(out=sc3[:, 0:16, :],
                                        in0=h24[:, :, 0, :],
                                        in1=h24[:, :, 1, :], op=ADD)
                nc.vector.tensor_reduce(
                    out=sc3[:, 16:17, :],
                    in_=ws3[:, 128:132, :].rearrange("p r t -> p t r"),
                    axis=X, op=ADD)

                # Top-72 per partition: 9 rounds of DVE max8 + match_replace.
                # match_replace replaces the first unmatched occurrence, so
                # ties resolve to the lowest index like jax.lax.top_k.
                for it in range(KEEP // 8):
                    m8 = smallp.tile([128, 8], f32,
                                     name=f"m8_g{g}i{it}", tag="m8")
                    nc.vector.max(out=m8[:, :], in_=scores[:, :])
                    nc.vector.match_replace(out=scores[:, :],
                                            in_to_replace=m8[:, :],
                                            in_values=scores[:, :],
                                            imm_value=NEG)

                # Ordering dep: a broadcast copy over the next group's wsum
                # ranges of the DVE-pooled chunks (2-4) so the scheduler
                # cannot start the next group's big W-pool reduces before
                # this group's topk has finished. The GpSimd-pooled ranges
                # (chunks 0-1) stay unconstrained on purpose.
                if g + 1 < ngroups:
                    nc.vector.tensor_copy(
                        out=wsums[g + 1][0:1, 1088:2177],
                        in_=scores[0:1, 288:289].broadcast_to((1, 1089)))

                # Mask + W-expand on Scalar: relu(score * -2^-100) is 1.0
                # exactly for replaced (top-72) entries, 0.0 otherwise.
                # One op per row range over the [17 blocks x 8] layout
                # (cols 132:136 get written too; they are never read).
                # Split so the rows chunk 0 needs are produced first.
                wexp = onep.tile([128, NB * WE], f32, name=f"wexp_g{g}",
                                 tag="wexp")
                we3 = wexp.rearrange("p (h w) -> p h w", w=WE)

                def wexp_rows(h0, h1):
                    outW = (we3[:, h0:h1, :]
                            .rearrange("p h (t c) -> p h t c", c=8))
                    inW = (sc3[:, h0:h1, :].unsqueeze(3)
                           .broadcast_to((128, h1 - h0, NB, 8)))
                    nc.scalar.activation(out=outW, in_=inW, func=RELU,
                                         scale=MSCALE)

                # Upsample 8x in H in place over the feature chunks (one 3D
                # stride-0 broadcast copy on Scalar per chunk), then store
                # from the Scalar engine's own HWDGE queue.
                wexp_rows(0, 4)
                for k, (r0, r1) in enumerate(ROW_CHUNKS):
                    if k == 1:
                        wexp_rows(4, NB)
                    ch = chunks[k]
                    v = ch.rearrange("p (r w) -> p r w", w=W)
                    hg0 = r0 // 8
                    nfull = (min(r1, 128) - r0) // 8
                    nr = nfull * 8
                    if nfull:
                        outA = v[:, 0:nr, :].rearrange(
                            "p (h r) w -> p h r w", r=8)
                        inA = (we3[:, hg0:hg0 + nfull, 0:W].unsqueeze(2)
                               .broadcast_to((128, nfull, 8, W)))
                        nc.scalar.copy(out=outA, in_=inA)
                    if r1 > 128:  # rows 128..131: the 4-row partial hgroup
                        a = max(0, 128 - r0)
                        outB = v[:, a:a + 4, :]
                        inB = we3[:, 16:17, 0:W].broadcast_to((128, 4, W))
                        nc.scalar.copy(out=outB, in_=inB)
                    nc.scalar.dma_start(out=y[p0:p0 + 128, r0 * W:r1 * W],
                                        in_=ch[:, :])
    nc.compile()
    return nc


def _ensure_ntff_hook_module():
    """bass_utils' trace path does `from antenv.axon_hooks import
    get_axon_ntff_profile_hook` — a module this image doesn't ship.
    Register an equivalent (ctypes into libaxon_pjrt.so, mirroring
    trn_boot._ntff_profile_via_ctypes) so BASS_TRACE=1 works; degrade
    to a None hook (trace skipped) when unavailable."""
    import sys
    import types

    try:
        import antenv.axon_hooks  # noqa: F401
        return
    except Exception:
        pass

    hook = None
    try:
        import contextlib
        import ctypes

        so_path = "/opt/axon/libaxon_pjrt.so"
        lib = ctypes.CDLL(so_path)
        if hasattr(lib, "axon_start_nrt_profile"):
            lib.axon_start_nrt_profile.argtypes = [
                ctypes.POINTER(ctypes.c_int64), ctypes.c_size_t]
            lib.axon_start_nrt_profile.restype = ctypes.c_int64
            lib.axon_stop_nrt_profile.argtypes = [ctypes.c_char_p]
            lib.axon_stop_nrt_profile.restype = ctypes.c_int64

            @contextlib.contextmanager
            def _hook(output_dir, device_ids):
                import jax
                jax.devices()
                if device_ids:
                    ids = (ctypes.c_int64 * len(device_ids))(*device_ids)
                    rc = lib.axon_start_nrt_profile(ids, len(device_ids))
                else:
                    rc = lib.axon_start_nrt_profile(None, 0)
                if rc != 0:
                    raise RuntimeError(f"axon_start_nrt_profile rc={rc}")
                try:
                    yield
                finally:
                    n = lib.axon_stop_nrt_profile(str(output_dir).encode())
                    print(f"ntff profile: {n} file(s) -> {output_dir}",
                          file=sys.stderr)

            hook = _hook
    except Exception:
        hook = None

    mod = types.ModuleType("antenv.axon_hooks")
    mod.get_axon_ntff_profile_hook = lambda: hook
    mod.set_axon_ntff_profile_hook = lambda h: None
    sys.modules["antenv.axon_hooks"] = mod


def _get_program():
    if "nc" not in _prog_cache:
        _prog_cache["nc"] = _build_program()
    return _prog_cache["nc"]


def kernel(features, enabled):
    feats = np.asarray(features)
    if not bool(np.asarray(enabled)):
        return np.ones(feats.shape, dtype=np.float32)

    _ensure_ntff_hook_module()
    import concourse.bass_utils as _bu
    from concourse.bass_utils import run_bass_kernel_spmd

    # The trace path uploads artifacts to a shared bucket; tolerate
    # sandboxes where that fails.
    if not getattr(_bu, "_upload_patched", False):
        _orig_upload = _bu.upload_artifacts

        def _safe_upload(tmpdir):
            try:
                return _orig_upload(tmpdir)
            except Exception:
                return str(tmpdir)

        _bu.upload_artifacts = _safe_upload
        _bu._upload_patched = True

    nc = _get_program()
    flat = np.ascontiguousarray(feats.reshape(B * C, HW), dtype=np.float32)
    in_maps = [{"x": flat[i * S:(i + 1) * S]} for i in range(N_CORES)]
    res = run_bass_kernel_spmd(nc, in_maps, list(range(N_CORES)))
    _prog_cache["last_res"] = res
    out = np.concatenate([np.asarray(res.results[i]["y"])
                          for i in range(N_CORES)], axis=0)
    return out.reshape(B, C, H, W).astype(np.float32)


# revision 14
# speedup vs baseline: 1.1579x; 1.0629x over previous
"""ChannelBlockImportanceGate kernel for 8 Trainium2 NeuronCores.

Computes, per (b, c) slice of features [8, 256, 132, 132] f32:
  scores = block-sum of |x| over 8x8 blocks (17x17 grid, zero-padded edges)
  top-72 blocks (ties -> lowest index, matching jax.lax.top_k)
  output = per-pixel {0,1} mask upsampled 8x8 (cropped to 132x132)

The straight-through soft term of the reference cancels in the forward
pass (hard - sg(soft) + soft == hard up to ~1ulp), so the output is the
hard mask.

Sharding: purely data parallel. 2048 (b,c) slices -> 256 per core.
Per core: 2 groups of 128 slices; each slice occupies one SBUF
partition so pooling/topk are per-partition ops with no cross-partition
traffic. Top-72 uses 9 rounds of DVE max8 + match_replace(-2^100).

v5 structure (trace-driven; the Vector engine's serial chain paces the
second group's stores, so work is spread over all four engines):
 - Five row chunks (4x32 + 1x4 rows) so the last-loaded chunk's W-pool
   is 0.75us, shortening last-load -> last-group-mask latency.
 - Chunks 0-1 of each group take an offload W-pool path: Scalar does
   an in-place ACTIVATE Abs over the chunk, GpSimd runs the 8->1
   column-sum as a 3-level tensor_tensor add tree (the Pool engine ISA
   has no abs or reduce, and runs ~7.4us/chunk vs DVE's 4.4us — but in
   parallel). Chunks 2-4 stay on DVE tensor_reduce(|x|). This takes
   ~9us/group off the Vector critical chain.
 - H-pool is a 3-level tensor_tensor add tree (~2.7us) instead of one
   strided 8:1 tensor_reduce (3.8us).
 - A broadcast copy from group g's scores into group g+1's wsum (only
   the DVE-pooled chunk ranges) right after topk gives the Tile
   list-scheduler a dependency that stops it hoisting g+1's 4.4us
   W-pool reduces into g's topk max8/match_replace latency chain (in
   the v1 trace that delayed the first store by ~25us, leaving DMA
   idle 61->83us). The GpSimd-pooled ranges stay unconstrained — that
   engine is otherwise idle, so early execution there is free.
 - match_replace writes -2^100, so mask-and-W-expand is ONE Scalar
   ACTIVATE per row range: relu(score * -2^-100) is exactly 1.0 for
   replaced (top-72) entries and exactly 0.0 otherwise, written
   directly as a W-expanded row strip [128, 17, 136] via a stride-0
   broadcast read. Vector's post-topk work is just the dep copy.
 - Upsample 8x in H in place over a feature chunk = one 3D stride-0
   broadcast Scalar copy per chunk; each chunk's store is issued right
   afterwards from the Scalar engine's own HWDGE queue (loads live on
   the Sync queue, so a store waiting for its mask can never
   head-of-line-block a load). Store descriptors stay at the fat
   16.9KB-per-partition line-rate size.
"""

import numpy as np

B, C, H, W = 8, 256, 132, 132
HW = H * W            # 17424
NB = 17               # 8x8 blocks per side (132 padded to 136)
NBLK = NB * NB        # 289
KEEP = 72             # round(289 * 0.25)
N_CORES = 8
S = (B * C) // N_CORES  # 256 slices per core
ROW_CHUNKS = ((0, 32), (32, 64), (64, 96), (96, 128), (128, 132))
NEG = -(2.0 ** 100)
MSCALE = -(2.0 ** -100)
WE = NB * 8           # 136 cols per W-expanded row strip

_prog_cache = {}


def _build_program():
    import concourse.bacc as bacc
    import concourse.mybir as mybir
    import concourse.tile as tile

    f32 = mybir.dt.float32
    X = mybir.AxisListType.X
    ADD = mybir.AluOpType.add
    RELU = mybir.ActivationFunctionType.Relu
    ABS = mybir.ActivationFunctionType.Abs

    nc = bacc.Bacc("TRN2", debug=False, num_devices=N_CORES)
    x = nc.dram_tensor("x", (S, HW), f32, kind="ExternalInput")
    y = nc.dram_tensor("y", (S, HW), f32, kind="ExternalOutput")

    ngroups = S // 128

    with tile.TileContext(nc) as tc:
        with (
            tc.tile_pool(name="big", bufs=2) as bigp,
            tc.tile_pool(name="med", bufs=2) as medp,
            tc.tile_pool(name="one", bufs=1) as onep,
            tc.tile_pool(name="small", bufs=2) as smallp,
        ):
            # wsum tiles for all groups upfront so group g's epilogue can
            # write the ordering dep into group g+1's tile.
            wsums = [medp.tile([128, H * NB], f32, name=f"wsum_g{g}",
                               tag="wsum") for g in range(ngroups)]
            # gpsimd add-tree scratch, shared by all offloaded chunks (the
            # trees run back to back on the Pool engine's FIFO anyway).
            t1 = onep.tile([128, 32 * 16 * 4], f32, name="t1", tag="t1")
            t2 = onep.tile([128, 32 * 16 * 2], f32, name="t2", tag="t2")
            t13 = t1.rearrange("p (r q c) -> p r q c", q=16, c=4)
            t23 = t2.rearrange("p (r q c) -> p r q c", q=16, c=2)
            for g in range(ngroups):
                p0 = g * 128
                chunks = []
                for k, (r0, r1) in enumerate(ROW_CHUNKS):
                    ch = bigp.tile([128, (r1 - r0) * W], f32,
                                   name=f"ch_g{g}k{k}", tag=f"chunk{k}")
                    nc.sync.dma_start(out=ch[:, :],
                                      in_=x[p0:p0 + 128, r0 * W:r1 * W])
                    chunks.append(ch)

                # W-pool: per image row, |x| summed over 8-col groups
                # (16 full groups + one 4-col partial group).
                # Chunks 0-1: Scalar abs (in place) + GpSimd add tree.
                # Chunks 2-4: DVE tensor_reduce with the abs flag.
                wsum = wsums[g]
                ws3 = wsum.rearrange("p (r t) -> p r t", t=NB)
                for k, (r0, r1) in enumerate(ROW_CHUNKS):
                    v = chunks[k].rearrange("p (r w) -> p r w", w=W)
                    if k < 2 and g == ngroups - 1:
                        # Offload path, last group only: its pools run after
                        # the loads are mostly done, so the GpSimd SBUF
                        # traffic (~480GB/s during a tree) doesn't steal
                        # load bandwidth the way first-group offload did.
                        # Abs only cols 0:128 (what the tree consumes), so
                        # the 4-col partial reduce below reads untouched
                        # data and needs no cross-engine dependency.
                        nc.scalar.activation(out=v[:, :, 0:128],
                                             in_=v[:, :, 0:128], func=ABS)
                        a4 = v[:, :, 0:128].rearrange(
                            "p r (q c) -> p r q c", c=8)
                        nc.gpsimd.tensor_tensor(
                            out=t13, in0=a4[:, :, :, 0:4],
                            in1=a4[:, :, :, 4:8], op=ADD)
                        nc.gpsimd.tensor_tensor(
                            out=t23, in0=t13[:, :, :, 0:2],
                            in1=t13[:, :, :, 2:4], op=ADD)
                        nc.gpsimd.tensor_tensor(
                            out=ws3[:, r0:r1, 0:16], in0=t23[:, :, :, 0],
                            in1=t23[:, :, :, 1], op=ADD)
                    else:
                        nc.vector.tensor_reduce(
                            out=ws3[:, r0:r1, 0:16],
                            in_=v[:, :, 0:128].rearrange(
                                "p r (q c) -> p r q c", c=8),
                            axis=X, op=ADD, apply_absolute_value=True)
                    nc.vector.tensor_reduce(
                        out=ws3[:, r0:r1, 16:17],
                        in_=v[:, :, 128:132],
                        axis=X, op=ADD, apply_absolute_value=True)

                # H-pool: sum row sums over 8-row groups. Full 128 rows as a
                # 3-level pairwise add tree, the 4-row remainder reduced
                # directly -> scores [128, 289], layout h*17 + w.
                ht1 = onep.tile([128, 64 * NB], f32, name=f"ht1_g{g}",
                                tag="ht1")
                ht2 = onep.tile([128, 32 * NB], f32, name=f"ht2_g{g}",
                                tag="ht2")
                scores = smallp.tile([128, NBLK], f32,
                                     name=f"scores_g{g}", tag="scores")
                sc3 = scores.rearrange("p (h t) -> p h t", t=NB)
                w4 = ws3[:, 0:128, :].rearrange("p (a b) t -> p a b t", b=2)
                h13 = ht1.rearrange("p (r t) -> p r t", t=NB)
                nc.vector.tensor_tensor(out=h13[:, :, :],
                                        in0=w4[:, :, 0, :],
                                        in1=w4[:, :, 1, :], op=ADD)
                h14 = h13.rearrange("p (a b) t -> p a b t", b=2)
                h23 = ht2.rearrange("p (r t) -> p r t", t=NB)
                nc.vector.tensor_tensor(out=h23[:, :, :],
                                        in0=h14[:, :, 0, :],
                                        in1=h14[:, :, 1, :], op=ADD)
                h24 = h23.rearrange("p (a b) t -> p a b t", b=2)
                nc.vector.tensor_tensor(out=sc3[:, 0:16, :],
                                        in0=h24[:, :, 0, :],
                                        in1=h24[:, :, 1, :], op=ADD)
                nc.vector.tensor_reduce(
                    out=sc3[:, 16:17, :],
                    in_=ws3[:, 128:132, :].rearrange("p r t -> p t r"),
                    axis=X, op=ADD)

                # Top-72 per partition: 9 rounds of DVE max8 + match_replace.
                # match_replace replaces the first unmatched occurrence, so
                # ties resolve to the lowest index like jax.lax.top_k.
                for it in range(KEEP // 8):
                    m8 = smallp.tile([128, 8], f32,
                                     name=f"m8_g{g}i{it}", tag="m8")
                    nc.vector.max(out=m8[:, :], in_=scores[:, :])
                    nc.vector.match_replace(out=scores[:, :],
                                            in_to_replace=m8[:, :],
                                            in_values=scores[:, :],
                                            imm_value=NEG)

                # Ordering dep: a broadcast copy over the next group's wsum
                # ranges of the DVE-pooled chunks (2-4) so the scheduler
                # cannot start the next group's big W-pool reduces before
                # this group's topk has finished. The GpSimd-pooled ranges
                # (chunks 0-1) stay unconstrained on purpose.
                if g + 1 < ngroups:
                    for off in (1088, 1632, 2176):
                        nc.vector.tensor_copy(
                            out=wsums[g + 1][0:1, off:off + 1],
                            in_=scores[0:1, 288:289])

                # Mask + W-expand on Scalar: relu(score * -2^-100) is 1.0
                # exactly for replaced (top-72) entries, 0.0 otherwise.
                # One op per row range over the [17 blocks x 8] layout
                # (cols 132:136 get written too; they are never read).
                # Split so the rows chunk 0 needs are produced first.
                wexp = onep.tile([128, NB * WE], f32, name=f"wexp_g{g}",
                                 tag="wexp")
                we3 = wexp.rearrange("p (h w) -> p h w", w=WE)

                def wexp_rows(h0, h1):
                    outW = (we3[:, h0:h1, :]
                            .rearrange("p h (t c) -> p h t c", c=8))
                    inW = (sc3[:, h0:h1, :].unsqueeze(3)
                           .broadcast_to((128, h1 - h0, NB, 8)))
                    nc.scalar.activation(out=outW, in_=inW, func=RELU,
                                         scale=MSCALE)

                # Upsample 8x in H in place over the feature chunks (one 3D
                # stride-0 broadcast copy on Scalar per piece), then store
                # from the Scalar engine's own HWDGE queue. Chunk 0 goes in
                # two 16-row halves so the first store hits the (by now
                # idle) DMA rings ~2us after the mask exists; the wexp
                # pieces are emitted just before their first consumer.
                def upsample_store(k, r0, r1):
                    ch = chunks[k]
                    v = ch.rearrange("p (r w) -> p r w", w=W)
                    a0 = r0 - ROW_CHUNKS[k][0]
                    hg0 = r0 // 8
                    nfull = (min(r1, 128) - r0) // 8
                    nr = nfull * 8
                    if nfull:
                        outA = v[:, a0:a0 + nr, :].rearrange(
                            "p (h r) w -> p h r w", r=8)
                        inA = (we3[:, hg0:hg0 + nfull, 0:W].unsqueeze(2)
                               .broadcast_to((128, nfull, 8, W)))
                        nc.scalar.copy(out=outA, in_=inA)
                    if r1 > 128:  # rows 128..131: the 4-row partial hgroup
                        a = a0 + max(0, 128 - r0)
                        outB = v[:, a:a + 4, :]
                        inB = we3[:, 16:17, 0:W].broadcast_to((128, 4, W))
                        nc.scalar.copy(out=outB, in_=inB)
                    nc.scalar.dma_start(
                        out=y[p0:p0 + 128, r0 * W:r1 * W],
                        in_=ch[:, (a0 * W):(a0 + (r1 - r0)) * W])

                wexp_rows(0, 4)
                upsample_store(0, 0, 16)
                upsample_store(0, 16, 32)
                wexp_rows(4, 8)
                upsample_store(1, 32, 64)
                wexp_rows(8, NB)
                for k in (2, 3, 4):
                    upsample_store(k, *ROW_CHUNKS[k])
    nc.compile()
    return nc


def _ensure_ntff_hook_module():
    """bass_utils' trace path does `from antenv.axon_hooks import
    get_axon_ntff_profile_hook` — a module this image doesn't ship.
    Register an equivalent (ctypes into libaxon_pjrt.so, mirroring
    trn_boot._ntff_profile_via_ctypes) so BASS_TRACE=1 works; degrade
    to a None hook (trace skipped) when unavailable."""
    import sys
    import types

    try:
        import antenv.axon_hooks  # noqa: F401
        return
    except Exception:
        pass

    hook = None
    try:
        import contextlib
        import ctypes

        so_path = "/opt/axon/libaxon_pjrt.so"
        lib = ctypes.CDLL(so_path)
        if hasattr(lib, "axon_start_nrt_profile"):
            lib.axon_start_nrt_profile.argtypes = [
                ctypes.POINTER(ctypes.c_int64), ctypes.c_size_t]
            lib.axon_start_nrt_profile.restype = ctypes.c_int64
            lib.axon_stop_nrt_profile.argtypes = [ctypes.c_char_p]
            lib.axon_stop_nrt_profile.restype = ctypes.c_int64

            @contextlib.contextmanager
            def _hook(output_dir, device_ids):
                import jax
                jax.devices()
                if device_ids:
                    ids = (ctypes.c_int64 * len(device_ids))(*device_ids)
                    rc = lib.axon_start_nrt_profile(ids, len(device_ids))
                else:
                    rc = lib.axon_start_nrt_profile(None, 0)
                if rc != 0:
                    raise RuntimeError(f"axon_start_nrt_profile rc={rc}")
                try:
                    yield
                finally:
                    n = lib.axon_stop_nrt_profile(str(output_dir).encode())
                    print(f"ntff profile: {n} file(s) -> {output_dir}",
                          file=sys.stderr)

            hook = _hook
    except Exception:
        hook = None

    mod = types.ModuleType("antenv.axon_hooks")
    mod.get_axon_ntff_profile_hook = lambda: hook
    mod.set_axon_ntff_profile_hook = lambda h: None
    sys.modules["antenv.axon_hooks"] = mod


def _get_program():
    if "nc" not in _prog_cache:
        _prog_cache["nc"] = _build_program()
    return _prog_cache["nc"]


def kernel(features, enabled):
    feats = np.asarray(features)
    if not bool(np.asarray(enabled)):
        return np.ones(feats.shape, dtype=np.float32)

    _ensure_ntff_hook_module()
    import concourse.bass_utils as _bu
    from concourse.bass_utils import run_bass_kernel_spmd

    # The trace path uploads artifacts to a shared bucket; tolerate
    # sandboxes where that fails.
    if not getattr(_bu, "_upload_patched", False):
        _orig_upload = _bu.upload_artifacts

        def _safe_upload(tmpdir):
            try:
                return _orig_upload(tmpdir)
            except Exception:
                return str(tmpdir)

        _bu.upload_artifacts = _safe_upload
        _bu._upload_patched = True

    nc = _get_program()
    flat = np.ascontiguousarray(feats.reshape(B * C, HW), dtype=np.float32)
    in_maps = [{"x": flat[i * S:(i + 1) * S]} for i in range(N_CORES)]
    res = run_bass_kernel_spmd(nc, in_maps, list(range(N_CORES)))
    _prog_cache["last_res"] = res
    out = np.concatenate([np.asarray(res.results[i]["y"])
                          for i in range(N_CORES)], axis=0)
    return out.reshape(B, C, H, W).astype(np.float32)
